# revision 2
# baseline (speedup 1.0000x reference)
"""Trainium2 Bass kernel v5 for the 16-head attention block (d_model=1024,
seq=4096), tensor-parallel over heads across 8 cores (2 heads/core).

Dataflow highlights:
  - x / Wqkv / Wo are shipped fp16 (11-bit mantissa ~ fp32r precision for
    these magnitudes); all PE matmuls run at 1.0 cycles/row.
  - q/k are rms-normalized on DVE (rsqrt via pow(-0.5)), cast to fp16, and
    transposed by the DMA XBAR (SBUF->SBUF, ~14ns/tile) -- no PE transposes,
    no PSUM merge traffic. wq*wk (a diagonal metric inside the q.k
    contraction) is folded onto the transposed q rows by the Pool engine.
  - scores: S^T[k,q] fp16 matmuls (free=512); exp on ACT -> bf16 probs;
    roughly every third exp batch is computed on DVE instead via the
    Schraudolph bit-trick (round(S*log2e*2^7 + (127-c)*2^7) as int16 IS
    bf16 exp) so ACT stays below the PE roofline.
  - PV: z[q,65] per head accumulated over k-chunks (lhsT = probs slices,
    rhs = V'[k,65] bf16 with fused ones-column denominators).
  - normalize on DVE -> fp16 z_n, DMA-transposed into z_nT[d,s]; O-proj
    fp16; partial outputs summed on the host (the TP all-reduce).
  - q-tile 0's attention is zigzagged into phase 1; every other q-tile's
    normalize/O-proj is deferred into the next q-tile's batch loop.
"""

import numpy as np
from contextlib import ExitStack

import concourse.bass as bass
import concourse.tile as tile
from concourse import mybir

F32 = mybir.dt.float32
F16 = mybir.dt.float16
BF16 = mybir.dt.bfloat16
I16 = mybir.dt.int16
AF = mybir.ActivationFunctionType
ALU = mybir.AluOpType

D_MODEL = 1024
SEQ = 4096
N_HEADS = 16
D_HEAD = 64
N_CORES = 8
HEADS_LOCAL = 2
P = 128
F_LOCAL = 3 * HEADS_LOCAL * D_HEAD       # 384: [q0|q1|k0|k1|v0|v1]
DM_AUG = D_MODEL + P                     # 1152 rows: x^T plus ones-row block
NCH = DM_AUG // P                        # 9 contraction chunks
SB = SEQ // P                            # 32 s-blocks
QT = 8                                   # q-tiles of 512
QW = SEQ // QT                           # 512
QSUB = QW // P                           # 4 q-subblocks of 128 per q-tile
KB = SEQ // P                            # 32 k-blocks
EXP_BATCH = 2
EPS = 1e-6
# Schraudolph exp constants: int16(S*A + B) bitcast to bf16 ~= exp(S).
# +0.5 makes float->int truncation behave as round-to-nearest; |S| <= 64
# (Cauchy-Schwarz on rms-normalized 64-vectors) keeps y in (0, 32767).
SCHRAUD_A = float(np.log2(np.e) * 128.0)
SCHRAUD_B = float((127.0 - 0.0439) * 128.0 + 0.5)

MAX_WAITS = 1


def _split_excess_waits(nc):
    """This walrus build rejects instructions carrying more than one sync-wait
    command. Rewrite every instruction with more than MAX_WAITS waits into a
    chain of same-engine NoOps each carrying MAX_WAITS waits."""
    import bass_rust

    n_new = 0
    for f in nc.m.functions:
        for bb in f.blocks:
            changed = False
            out = []
            for ins in bb.instructions:
                si = ins.sync_info
                waits = list(si.on_wait) if si is not None and si.on_wait else []
                if len(waits) > MAX_WAITS:
                    changed = True
                    ncar = len(waits) - MAX_WAITS
                    for i in range(0, ncar, MAX_WAITS):
                        chunk = waits[i : min(i + MAX_WAITS, ncar)]
                        nop = mybir.InstNoOp(
                            name=f"{ins.name}-wsplit{i}", ins=[], outs=[]
                        )
                        nop.engine = ins.engine
                        nop.sync_info = bass_rust.SyncInfo(
                            on_wait=chunk, on_update=[]
                        )
                        out.append(nop)
                        n_new += 1
                    ins.sync_info = bass_rust.SyncInfo(
                        on_wait=waits[ncar:], on_update=si.on_update
                    )
                out.append(ins)
            if changed:
                bb.instructions = out
    return n_new


def build_core_kernel(split_waits=True, schraud_qt0=True, schraud_p2=True):
    nc = bass.Bass()
    xta = nc.declare_dram_parameter("xta", [DM_AUG, SEQ], F16, isOutput=False)
    wqkvt = nc.declare_dram_parameter("wqkvt", [DM_AUG, F_LOCAL], F16, isOutput=False)
    wot = nc.declare_dram_parameter("wot", [P, D_MODEL], F16, isOutput=False)
    wqwk = nc.declare_dram_parameter("wqwk", [P, 1], F32, isOutput=False)
    out = nc.declare_dram_parameter("out", [SEQ, D_MODEL], F32, isOutput=True)

    xta_r = xta.rearrange("(c p) s -> p c s", p=P)       # [128, 9, 4096]
    wqkvt_r = wqkvt.rearrange("(c p) f -> p c f", p=P)   # [128, 9, 384]

    with ExitStack() as ctx:
        tc = ctx.enter_context(tile.TileContext(nc))

        const = ctx.enter_context(tc.tile_pool(name="const", bufs=1))
        persist = ctx.enter_context(tc.tile_pool(name="persist", bufs=1))
        # zps PSUM slot shared by all q-tiles ([q(128), qsub, 256-pad] fp32,
        # 2 banks); per head the 65 columns at h*65 hold [z | denominator]
        zpool = ctx.enter_context(tc.tile_pool(name="zps", bufs=1, space="PSUM"))

        eps_t = const.tile([P, 1], F32)
        nc.gpsimd.memset(eps_t[:], EPS)
        # const DMAs go out on the ACT queue so the SP queue starts on x
        wqwk_sb = const.tile([P, 1], F32)
        nc.scalar.dma_start(wqwk_sb[:], wqwk[:])
        wqkv_sb = const.tile([P, NCH, F_LOCAL], F16)
        for c in range(NCH):
            nc.scalar.dma_start(wqkv_sb[:, c, :], wqkvt_r[:, c, :])
        wot_sb = const.tile([P, D_MODEL], F16)
        nc.scalar.dma_start(wot_sb[:], wot[:])

        # transposed q/k: rows (head, d), fp16
        qT2 = persist.tile([P, SEQ], F16)
        kT2 = persist.tile([P, SEQ], F16)
        # V' per head: [k(128), head, kb, 64+1] bf16 with ones column
        vp = persist.tile([P, HEADS_LOCAL, KB, D_HEAD + 1], BF16)
        nc.gpsimd.memset(vp[:, :, :, D_HEAD : D_HEAD + 1], 1.0)
        z_nT = persist.tile([P, SEQ], F16)
        recip = persist.tile([P, 2], F32)
        ss_all = persist.tile([P, SB, 4], F32)
        rs_all = persist.tile([P, SB, 4], F32)
        rr_all = persist.tile([P, SB, 4], F32)
        # two probs retention rings (one per q-tile parity): slot = kb*2 + h
        rings = [
            persist.tile([P, 2 * KB, QW], BF16, name=f"ring{i}") for i in range(2)
        ]

        zps_tiles = {}

        def get_zps(qt):
            t = zpool.tile([P, QSUB, 256], F32, name=f"zps{qt}", tag="zps")
            zps_tiles[qt] = t
            return t

        # ---------------- phase 1 (+ q-tile 0 zigzag) ----------------
        with ExitStack() as p1:
            xpool = p1.enter_context(tc.tile_pool(name="xt", bufs=3))
            norm = p1.enter_context(tc.tile_pool(name="norm", bufs=3))
            qkps = p1.enter_context(tc.tile_pool(name="qkvps", bufs=2, space="PSUM"))
            sp0 = p1.enter_context(tc.tile_pool(name="sps0", bufs=2, space="PSUM"))

            xt_queue = []

            def prefetch_x(sb):
                xt0 = xpool.tile([P, NCH, P], F16)
                nc.sync.dma_start(xt0[:], xta_r[:, :, bass.ts(sb, P)])
                xt_queue.append(xt0)

            def pass1(sb):
                ssl = bass.ts(sb, P)
                xt = xt_queue.pop(0)
                qkv_ps = qkps.tile([P, F_LOCAL], F32)
                for c in range(NCH):
                    nc.tensor.matmul(
                        qkv_ps[:],
                        lhsT=xt[:, c, :],
                        rhs=wqkv_sb[:, c, :],
                        start=(c == 0),
                        stop=(c == NCH - 1),
                    )
                qk_ps = qkv_ps[:, 0 : 4 * D_HEAD].rearrange("p (g d) -> p g d", g=4)
                sq = norm.tile([P, 4, D_HEAD], F32)
                nc.scalar.activation(sq[:], qk_ps, AF.Square)
                nc.vector.tensor_reduce(
                    ss_all[:, sb, :], sq[:], axis=mybir.AxisListType.X, op=ALU.add
                )
                # rs = sqrt(ss/64 + eps) on ACT: Square/Sqrt/Copy share one
                # activation table, and phase 1 runs no Exp (q-tile 0's exp
                # is Schraudolph on DVE), so the table never reloads
                nc.scalar.activation(
                    rs_all[:, sb, :],
                    ss_all[:, sb, :],
                    AF.Sqrt,
                    bias=eps_t[:],
                    scale=1.0 / D_HEAD,
                )
                nc.vector.reciprocal(rr_all[:, sb, :], rs_all[:, sb, :])
                qk_hat = norm.tile([P, 4, D_HEAD], F16)
                nc.vector.tensor_tensor(
                    qk_hat[:],
                    qk_ps,
                    rr_all[:, sb, :, None].to_broadcast((P, 4, D_HEAD)),
                    ALU.mult,
                )
                # V drain on ACT (Copy lives in the Exp table set)
                nc.scalar.activation(
                    vp[:, :, sb, 0:D_HEAD],
                    qkv_ps[:, 4 * D_HEAD : 6 * D_HEAD].rearrange(
                        "p (h d) -> p h d", h=2
                    ),
                    AF.Copy,
                )
                # XBAR transposes; then wq*wk onto the q rows (Pool)
                nc.sync.dma_start_transpose(
                    qT2[:, ssl], qk_hat[:, 0:2, :].rearrange("p g d -> p (g d)")
                )
                nc.sync.dma_start_transpose(
                    kT2[:, ssl], qk_hat[:, 2:4, :].rearrange("p g d -> p (g d)")
                )
                nc.gpsimd.tensor_scalar(
                    qT2[:, ssl], qT2[:, ssl], wqwk_sb[:], None, ALU.mult
                )

            def qt0_step(kb):
                sps0 = sp0.tile([P, HEADS_LOCAL, QW], F32, name="sps0", tag="sps0")
                for h in range(HEADS_LOCAL):
                    nc.tensor.matmul(
                        sps0[:, h, :],
                        lhsT=kT2[h * D_HEAD : (h + 1) * D_HEAD, bass.ts(kb, P)],
                        rhs=qT2[h * D_HEAD : (h + 1) * D_HEAD, 0:QW],
                        start=True,
                        stop=True,
                    )
                # q-tile 0's exp runs as Schraudolph-int16 on DVE so phase 1's
                # ACT stream (Square/Sqrt/Copy) never swaps tables with Exp
                nc.vector.tensor_scalar(
                    rings[0][:, 2 * kb : 2 * kb + 2, :].bitcast(I16),
                    sps0[:],
                    SCHRAUD_A,
                    SCHRAUD_B,
                    ALU.mult,
                    ALU.add,
                )

            for sb in range(3):
                prefetch_x(sb)
            for sb in range(SB):
                if sb + 3 < SB:
                    prefetch_x(sb + 3)
                pass1(sb)
                if sb >= 4:
                    qt0_step(sb - 4)
            # q-tile 0's last steps are deferred into q-tile 1's batch loop
            qt0_rest = list(range(28, KB))

        # ---------- phase 2: q-tiles 1..7 + deferred prev finalize ----------
        with ExitStack() as p2:
            spool = p2.enter_context(tc.tile_pool(name="sps", bufs=3, space="PSUM"))
            znpool = p2.enter_context(tc.tile_pool(name="zn", bufs=3))
            osb = p2.enter_context(tc.tile_pool(name="osb", bufs=3))

            def pv_wave(qt, qs, h):
                # one (qsub, head) accumulation group, 32 consecutive 65-wide
                # matmuls; groups sharing a PSUM bank never interleave (the
                # hardware marks the whole 2KB bank pending-zero on start)
                if qt not in zps_tiles:
                    get_zps(qt)
                zps = zps_tiles[qt]
                ring = rings[qt % 2]
                for kb in range(KB):
                    nc.tensor.matmul(
                        zps[:, qs, h * 65 : h * 65 + 65],
                        lhsT=ring[:, 2 * kb + h, bass.ts(qs, P)],
                        rhs=vp[:, h, kb, :],
                        start=(kb == 0),
                        stop=(kb == KB - 1),
                        skip_group_check=True,
                    )

            def norm_dve_qs(qt, qs):
                # normalize by the fused denominators; z_n fp16 is
                # XBAR-transposed straight into z_nT[d, s]
                zps = zps_tiles[qt]
                sb = qt * QSUB + qs
                nc.vector.reciprocal(recip[:, 0:2], zps[:, qs, 64 : 64 + 66 : 65])
                z_n = znpool.tile([P, 2, D_HEAD], F16, name="zn", tag="zn")
                for h in range(HEADS_LOCAL):
                    nc.vector.tensor_scalar(
                        z_n[:, h, :],
                        zps[:, qs, h * 65 : h * 65 + D_HEAD],
                        recip[:, h : h + 1],
                        None,
                        ALU.mult,
                    )
                nc.sync.dma_start_transpose(
                    z_nT[:, bass.ts(sb, P)], z_n[:].rearrange("p h d -> p (h d)")
                )

            def emit_oproj_sb(qt, sbl):
                sb = qt * QSUB + sbl
                ops = spool.tile([P, D_MODEL], F32, name="ops", tag="sps")
                for half in range(2):
                    nc.tensor.matmul(
                        ops[:, bass.ts(half, QW)],
                        lhsT=z_nT[:, bass.ts(sb, P)],
                        rhs=wot_sb[:, bass.ts(half, QW)],
                        start=True,
                        stop=True,
                    )
                ot = osb.tile([P, D_MODEL], F32, name="ot", tag="ot")
                nc.vector.tensor_copy(ot[:], ops[:])
                nc.sync.dma_start(out[bass.ts(sb, P), :], ot[:])

            def qt0_step_p2(kb):
                # q-tile 0's deferred last score/exp steps (ACT Exp is fine
                # here: phase 2's activation table is already Exp)
                sps = spool.tile([P, EXP_BATCH, QW], F32, name="sps", tag="sps")
                for h in range(HEADS_LOCAL):
                    nc.tensor.matmul(
                        sps[:, h, :],
                        lhsT=kT2[h * D_HEAD : (h + 1) * D_HEAD, bass.ts(kb, P)],
                        rhs=qT2[h * D_HEAD : (h + 1) * D_HEAD, 0:QW],
                        start=True,
                        stop=True,
                    )
                nc.scalar.activation(
                    rings[0][:, 2 * kb : 2 * kb + 2, :], sps[:], AF.Exp
                )

            # deferred-work queue: each q-tile's loop drains `pending` (prior
            # q-tile finalize) plus its own h0 PV waves, spread evenly
            pending = (
                [lambda kb=kb: qt0_step_p2(kb) for kb in qt0_rest]
                + [lambda qs=qs: pv_wave(0, qs, 0) for qs in range(QSUB)]
                + [lambda qs=qs: pv_wave(0, qs, 1) for qs in range(QSUB)]
                + [lambda qs=qs: norm_dve_qs(0, qs) for qs in range(QSUB)]
                + [lambda qs=qs: emit_oproj_sb(0, qs) for qs in range(QSUB)]
            )

            for qt in range(1, QT):
                prev = qt - 1
                qsl = bass.ts(qt, QW)
                ring = rings[qt % 2]
                # h-major tiles: h0's ring slots complete by mid-loop so this
                # q-tile's own h0 PV waves can run before the loop ends
                tiles = [(kb, h) for h in range(HEADS_LOCAL) for kb in range(KB)]
                items = pending + [
                    lambda qs=qs, qt=qt: pv_wave(qt, qs, 0) for qs in range(QSUB)
                ]
                own_start = len(pending)
                n_batches = (len(tiles) + EXP_BATCH - 1) // EXP_BATCH
                slots = {}
                for i in range(len(items)):
                    # own h0 waves must wait for this q-tile's h0 exps
                    lo = (
                        int(i * n_batches / len(items))
                        if i < own_start
                        else max(n_batches - 2 * (len(items) - i), n_batches // 2 + 2)
                    )
                    slots.setdefault(lo, []).append(items[i])

                for bidx, b0 in enumerate(range(0, len(tiles), EXP_BATCH)):
                    batch = tiles[b0 : b0 + EXP_BATCH]
                    nb = len(batch)
                    h = batch[0][1]
                    kb0 = batch[0][0]
                    sps = spool.tile([P, EXP_BATCH, QW], F32, name="sps", tag="sps")
                    for j, (kb, hh) in enumerate(batch):
                        nc.tensor.matmul(
                            sps[:, j, :],
                            lhsT=kT2[
                                hh * D_HEAD : (hh + 1) * D_HEAD, bass.ts(kb, P)
                            ],
                            rhs=qT2[hh * D_HEAD : (hh + 1) * D_HEAD, qsl],
                            start=True,
                            stop=True,
                        )
                    dst = ring[:, 2 * kb0 + h : 2 * (kb0 + nb - 1) + h + 1 : 2, :]
                    if schraud_p2 and bidx % 3 == 2:
                        # exp via int16 bit-trick on DVE (offloads ACT)
                        nc.vector.tensor_scalar(
                            dst.bitcast(I16),
                            sps[:, 0:nb, :],
                            SCHRAUD_A,
                            SCHRAUD_B,
                            ALU.mult,
                            ALU.add,
                        )
                    else:
                        nc.scalar.activation(dst, sps[:, 0:nb, :], AF.Exp)
                    for fn in slots.get(bidx, []):
                        fn()

                pending = (
                    [lambda qs=qs, p=qt: pv_wave(p, qs, 1) for qs in range(QSUB)]
                    + [lambda qs=qs, p=qt: norm_dve_qs(p, qs) for qs in range(QSUB)]
                    + [lambda qs=qs, p=qt: emit_oproj_sb(p, qs) for qs in range(QSUB)]
                )

            # tail: drain the last q-tile's finalize, interleaved per qsub so
            # the PE wave / DVE normalize / PE O-proj stages pipeline
            last = QT - 1
            pv_wave(last, 0, 1)
            for qs in range(QSUB):
                if qs + 1 < QSUB:
                    pv_wave(last, qs + 1, 1)
                norm_dve_qs(last, qs)
                emit_oproj_sb(last, qs)

    if split_waits:
        _split_excess_waits(nc)
    return nc


def shard_inputs(x, Wqkv, bqkv, Wo, bo, wq, wk):
    x2 = np.ascontiguousarray(np.asarray(x, dtype=np.float32).reshape(SEQ, D_MODEL))
    Wqkv = np.asarray(Wqkv, dtype=np.float32)
    bqkv = np.asarray(bqkv, dtype=np.float32)
    Wo = np.asarray(Wo, dtype=np.float32)
    wq = np.asarray(wq, dtype=np.float32)
    wk = np.asarray(wk, dtype=np.float32)

    xta = np.zeros((DM_AUG, SEQ), np.float16)
    xta[:D_MODEL] = x2.T.astype(np.float16)
    xta[D_MODEL] = 1.0
    xta = np.ascontiguousarray(xta)

    # wq*wk is a diagonal metric inside the q.k contraction; applied to the
    # transposed q rows [(q0 d), (q1 d)]
    w2 = (wq * wk).astype(np.float32)
    wqwk2 = np.concatenate([w2, w2]).reshape(P, 1)

    in_maps = []
    for c in range(N_CORES):
        rows, brows = [], []
        for part in range(3):
            for h in (HEADS_LOCAL * c, HEADS_LOCAL * c + 1):
                sl = slice(
                    part * D_MODEL + h * D_HEAD, part * D_MODEL + (h + 1) * D_HEAD
                )
                rows.append(Wqkv[sl])
                brows.append(bqkv[sl])
        Wl = np.concatenate(rows, 0)          # [384, 1024]
        bl = np.concatenate(brows, 0)         # [384]
        wqkvta = np.zeros((DM_AUG, F_LOCAL), np.float16)
        wqkvta[:D_MODEL] = Wl.T.astype(np.float16)
        wqkvta[D_MODEL] = bl.astype(np.float16)
        cols = slice(
            HEADS_LOCAL * c * D_HEAD, (HEADS_LOCAL * c + HEADS_LOCAL) * D_HEAD
        )
        wotc = np.ascontiguousarray(Wo[:, cols].T.astype(np.float16))  # [128, 1024]
        in_maps.append(
            {
                "xta": xta,
                "wqkvt": np.ascontiguousarray(wqkvta),
                "wot": wotc,
                "wqwk": wqwk2,
            }
        )
    return in_maps


_NC_CACHE = {}
LAST_RESULT = None


def kernel(x, Wqkv, bqkv, Wo, bo, wq, wk):
    import os
    from concourse.bass_utils import run_bass_kernel_spmd

    global LAST_RESULT
    assert np.asarray(x).shape == (1, SEQ, D_MODEL)
    in_maps = shard_inputs(x, Wqkv, bqkv, Wo, bo, wq, wk)
    if "nc" not in _NC_CACHE:
        _NC_CACHE["nc"] = build_core_kernel()
    nc = _NC_CACHE["nc"]
    trace = bool(int(os.environ.get("BASS_KERNEL_TRACE", "0")))
    res = run_bass_kernel_spmd(nc, in_maps, list(range(N_CORES)), trace=trace)
    LAST_RESULT = res
    acc = np.zeros((SEQ, D_MODEL), np.float64)
    for c in range(N_CORES):
        acc += res.results[c]["out"].astype(np.float64)
    acc += np.asarray(bo, dtype=np.float64)
    return acc.astype(np.float32).reshape(1, SEQ, D_MODEL)


# revision 3
# speedup vs baseline: 1.0752x; 1.0752x over previous
"""Trainium2 Bass kernel v5 for the 16-head attention block (d_model=1024,
seq=4096), tensor-parallel over heads across 8 cores (2 heads/core).

Dataflow highlights:
  - x / Wqkv / Wo are shipped fp16 (11-bit mantissa ~ fp32r precision for
    these magnitudes); all PE matmuls run at 1.0 cycles/row.
  - q/k are rms-normalized on DVE (rsqrt via pow(-0.5)), cast to fp16, and
    transposed by the DMA XBAR (SBUF->SBUF, ~14ns/tile) -- no PE transposes,
    no PSUM merge traffic. wq*wk (a diagonal metric inside the q.k
    contraction) is folded onto the transposed q rows by the Pool engine.
  - scores: S^T[k,q] fp16 matmuls (free=512); exp on ACT -> bf16 probs;
    roughly every third exp batch is computed on DVE instead via the
    Schraudolph bit-trick (round(S*log2e*2^7 + (127-c)*2^7) as int16 IS
    bf16 exp) so ACT stays below the PE roofline.
  - PV: z[q,65] per head accumulated over k-chunks (lhsT = probs slices,
    rhs = V'[k,65] bf16 with fused ones-column denominators).
  - normalize on DVE -> fp16 z_n, DMA-transposed into z_nT[d,s]; O-proj
    fp16; partial outputs summed on the host (the TP all-reduce).
  - q-tile 0's attention is zigzagged into phase 1; every other q-tile's
    normalize/O-proj is deferred into the next q-tile's batch loop.
"""

import numpy as np
from contextlib import ExitStack

import concourse.bass as bass
import concourse.tile as tile
from concourse import mybir

F32 = mybir.dt.float32
F16 = mybir.dt.float16
BF16 = mybir.dt.bfloat16
I16 = mybir.dt.int16
AF = mybir.ActivationFunctionType
ALU = mybir.AluOpType

D_MODEL = 1024
SEQ = 4096
N_HEADS = 16
D_HEAD = 64
N_CORES = 8
HEADS_LOCAL = 2
P = 128
F_LOCAL = 3 * HEADS_LOCAL * D_HEAD       # 384: [q0|q1|k0|k1|v0|v1]
DM_AUG = D_MODEL + P                     # 1152 rows: x^T plus ones-row block
NCH = DM_AUG // P                        # 9 contraction chunks
SB = SEQ // P                            # 32 s-blocks
QT = 8                                   # q-tiles of 512
QW = SEQ // QT                           # 512
QSUB = QW // P                           # 4 q-subblocks of 128 per q-tile
KB = SEQ // P                            # 32 k-blocks
EXP_BATCH = 2
EPS = 1e-6
# Schraudolph exp constants: int16(S*A + B) bitcast to bf16 ~= exp(S).
# +0.5 makes float->int truncation behave as round-to-nearest; |S| <= 64
# (Cauchy-Schwarz on rms-normalized 64-vectors) keeps y in (0, 32767).
SCHRAUD_A = float(np.log2(np.e) * 128.0)
SCHRAUD_B = float((127.0 - 0.0439) * 128.0 + 0.5)

MAX_WAITS = 1


def _split_excess_waits(nc):
    """This walrus build rejects instructions carrying more than one sync-wait
    command. Rewrite every instruction with more than MAX_WAITS waits into a
    chain of same-engine NoOps each carrying MAX_WAITS waits."""
    import bass_rust

    n_new = 0
    for f in nc.m.functions:
        for bb in f.blocks:
            changed = False
            out = []
            for ins in bb.instructions:
                si = ins.sync_info
                waits = list(si.on_wait) if si is not None and si.on_wait else []
                if len(waits) > MAX_WAITS:
                    changed = True
                    ncar = len(waits) - MAX_WAITS
                    for i in range(0, ncar, MAX_WAITS):
                        chunk = waits[i : min(i + MAX_WAITS, ncar)]
                        nop = mybir.InstNoOp(
                            name=f"{ins.name}-wsplit{i}", ins=[], outs=[]
                        )
                        nop.engine = ins.engine
                        nop.sync_info = bass_rust.SyncInfo(
                            on_wait=chunk, on_update=[]
                        )
                        out.append(nop)
                        n_new += 1
                    ins.sync_info = bass_rust.SyncInfo(
                        on_wait=waits[ncar:], on_update=si.on_update
                    )
                out.append(ins)
            if changed:
                bb.instructions = out
    return n_new


def build_core_kernel(split_waits=True, schraud_qt0=True, schraud_p2=True):
    nc = bass.Bass()
    # x is host-tiled per s-block so each partition's DMA line is one
    # contiguous 2304B run (strided 256B lines halve DMA throughput)
    xta = nc.declare_dram_parameter("xta", [SB, P, NCH, P], F16, isOutput=False)
    wqkvt = nc.declare_dram_parameter("wqkvt", [DM_AUG, F_LOCAL], F16, isOutput=False)
    wot = nc.declare_dram_parameter("wot", [P, D_MODEL], F16, isOutput=False)
    wqwk = nc.declare_dram_parameter("wqwk", [P, 1], F32, isOutput=False)
    out = nc.declare_dram_parameter("out", [SEQ, D_MODEL], F32, isOutput=True)

    wqkvt_r = wqkvt.rearrange("(c p) f -> p c f", p=P)   # [128, 9, 384]

    with ExitStack() as ctx:
        tc = ctx.enter_context(tile.TileContext(nc))

        const = ctx.enter_context(tc.tile_pool(name="const", bufs=1))
        persist = ctx.enter_context(tc.tile_pool(name="persist", bufs=1))

        eps_t = const.tile([P, 1], F32)
        nc.gpsimd.memset(eps_t[:], EPS)
        # touch the Exp table immediately so the 1.3us activation-table load
        # overlaps the initial DMAs instead of stalling the first rms op
        twarm = const.tile([P, 1], F32)
        nc.scalar.activation(twarm[:], eps_t[:], AF.Exp)
        # const DMAs go out on the ACT queue so the SP queue starts on x
        wqwk_sb = const.tile([P, 1], F32)
        nc.scalar.dma_start(wqwk_sb[:], wqwk[:])
        wqkv_sb = const.tile([P, NCH, F_LOCAL], F16)
        for c in range(NCH):
            nc.scalar.dma_start(wqkv_sb[:, c, :], wqkvt_r[:, c, :])
        wot_sb = const.tile([P, D_MODEL], F16)
        nc.scalar.dma_start(wot_sb[:], wot[:])

        # transposed q/k: rows (head, d), fp16
        qT2 = persist.tile([P, SEQ], F16)
        kT2 = persist.tile([P, SEQ], F16)
        # V' per head: [k(128), head, kb, 64+1] bf16 with ones column
        vp = persist.tile([P, HEADS_LOCAL, KB, D_HEAD + 1], BF16)
        nc.gpsimd.memset(vp[:, :, :, D_HEAD : D_HEAD + 1], 1.0)
        z_nT = persist.tile([P, SEQ], F16)
        recip = persist.tile([P, 2], F32)
        ss_all = persist.tile([P, SB, 4], F32)
        rs_all = persist.tile([P, SB, 4], F32)
        rr_all = persist.tile([P, SB, 4], F32)
        # two probs retention rings (one per q-tile parity): slot = kb*2 + h
        rings = [
            persist.tile([P, 2 * KB, QW], BF16, name=f"ring{i}") for i in range(2)
        ]

        zps_tiles = {}

        def get_zps(qt):
            t = zpool.tile([P, QSUB, 256], F32, name=f"zps{qt}", tag="zps")
            zps_tiles[qt] = t
            return t

        # ---------------- phase 1 (+ q-tile 0 zigzag) ----------------
        with ExitStack() as p1:
            xpool = p1.enter_context(tc.tile_pool(name="xt", bufs=3))
            norm = p1.enter_context(tc.tile_pool(name="norm", bufs=3))
            qkps = p1.enter_context(tc.tile_pool(name="qkvps", bufs=3, space="PSUM"))
            sp0 = p1.enter_context(tc.tile_pool(name="sps0", bufs=2, space="PSUM"))

            xt_queue = []

            def prefetch_x(sb):
                xt0 = xpool.tile([P, NCH, P], F16)
                nc.sync.dma_start(xt0[:], xta[sb])
                xt_queue.append(xt0)

            def pass1(sb):
                ssl = bass.ts(sb, P)
                xt = xt_queue.pop(0)
                qkv_ps = qkps.tile([P, F_LOCAL], F32)
                for c in range(NCH):
                    nc.tensor.matmul(
                        qkv_ps[:],
                        lhsT=xt[:, c, :],
                        rhs=wqkv_sb[:, c, :],
                        start=(c == 0),
                        stop=(c == NCH - 1),
                    )
                qk_ps = qkv_ps[:, 0 : 4 * D_HEAD].rearrange("p (g d) -> p g d", g=4)
                sq = norm.tile([P, 4, D_HEAD], F32)
                nc.scalar.activation(sq[:], qk_ps, AF.Square)
                nc.vector.tensor_reduce(
                    ss_all[:, sb, :], sq[:], axis=mybir.AxisListType.X, op=ALU.add
                )
                # rsqrt(ms + eps) = Exp(-0.5 * Ln(ms + eps)): Ln/Exp/Square/
                # Copy all live in the natural_log_exp activation table, so
                # the whole kernel runs on a single table load
                nc.scalar.activation(
                    rs_all[:, sb, :],
                    ss_all[:, sb, :],
                    AF.Ln,
                    bias=eps_t[:],
                    scale=1.0 / D_HEAD,
                )
                nc.scalar.activation(
                    rr_all[:, sb, :], rs_all[:, sb, :], AF.Exp, scale=-0.5
                )
                qk_hat = norm.tile([P, 4, D_HEAD], F16)
                nc.vector.tensor_tensor(
                    qk_hat[:],
                    qk_ps,
                    rr_all[:, sb, :, None].to_broadcast((P, 4, D_HEAD)),
                    ALU.mult,
                )
                # V drain on ACT (Copy lives in the Exp table set)
                nc.scalar.activation(
                    vp[:, :, sb, 0:D_HEAD],
                    qkv_ps[:, 4 * D_HEAD : 6 * D_HEAD].rearrange(
                        "p (h d) -> p h d", h=2
                    ),
                    AF.Copy,
                )
                # XBAR transposes; then wq*wk onto the q rows (Pool)
                nc.sync.dma_start_transpose(
                    qT2[:, ssl], qk_hat[:, 0:2, :].rearrange("p g d -> p (g d)")
                )
                nc.sync.dma_start_transpose(
                    kT2[:, ssl], qk_hat[:, 2:4, :].rearrange("p g d -> p (g d)")
                )
                nc.gpsimd.tensor_scalar(
                    qT2[:, ssl], qT2[:, ssl], wqwk_sb[:], None, ALU.mult
                )

            def qt0_step(kb):
                sps0 = sp0.tile([P, HEADS_LOCAL, QW], F32, name="sps0", tag="sps0")
                for h in range(HEADS_LOCAL):
                    nc.tensor.matmul(
                        sps0[:, h, :],
                        lhsT=kT2[h * D_HEAD : (h + 1) * D_HEAD, bass.ts(kb, P)],
                        rhs=qT2[h * D_HEAD : (h + 1) * D_HEAD, 0:QW],
                        start=True,
                        stop=True,
                    )
                # q-tile 0's exps split between ACT (same table as Ln) and
                # DVE Schraudolph to balance the phase-1 engine load
                if kb % 3 == 0:
                    nc.scalar.activation(
                        rings[0][:, 2 * kb : 2 * kb + 2, :], sps0[:], AF.Exp
                    )
                else:
                    nc.vector.tensor_scalar(
                        rings[0][:, 2 * kb : 2 * kb + 2, :].bitcast(I16),
                        sps0[:],
                        SCHRAUD_A,
                        SCHRAUD_B,
                        ALU.mult,
                        ALU.add,
                    )

            for sb in range(3):
                prefetch_x(sb)
            for sb in range(SB):
                if sb + 3 < SB:
                    prefetch_x(sb + 3)
                pass1(sb)
                if sb >= 4:
                    qt0_step(sb - 4)
            # q-tile 0's last steps are deferred into q-tile 1's batch loop
            qt0_rest = list(range(28, KB))

        # ---------- phase 2: q-tiles 1..7 + deferred prev finalize ----------
        with ExitStack() as p2:
            spool = p2.enter_context(tc.tile_pool(name="sps", bufs=3, space="PSUM"))
            # zps PSUM slot shared by all q-tiles ([q(128), qsub, 256-pad]
            # fp32, 2 banks); per head the 65 cols at h*65 = [z | denominator]
            zpool = p2.enter_context(tc.tile_pool(name="zps", bufs=1, space="PSUM"))
            znpool = p2.enter_context(tc.tile_pool(name="zn", bufs=3))
            osb = p2.enter_context(tc.tile_pool(name="osb", bufs=3))

            def pv_wave(qt, qs, h):
                # one (qsub, head) accumulation group, 32 consecutive 65-wide
                # matmuls; groups sharing a PSUM bank never interleave (the
                # hardware marks the whole 2KB bank pending-zero on start)
                if qt not in zps_tiles:
                    get_zps(qt)
                zps = zps_tiles[qt]
                ring = rings[qt % 2]
                for kb in range(KB):
                    nc.tensor.matmul(
                        zps[:, qs, h * 65 : h * 65 + 65],
                        lhsT=ring[:, 2 * kb + h, bass.ts(qs, P)],
                        rhs=vp[:, h, kb, :],
                        start=(kb == 0),
                        stop=(kb == KB - 1),
                        skip_group_check=True,
                    )

            def norm_dve_qs(qt, qs):
                # normalize by the fused denominators; z_n fp16 is
                # XBAR-transposed straight into z_nT[d, s]
                zps = zps_tiles[qt]
                sb = qt * QSUB + qs
                nc.vector.reciprocal(recip[:, 0:2], zps[:, qs, 64 : 64 + 66 : 65])
                z_n = znpool.tile([P, 2, D_HEAD], F16, name="zn", tag="zn")
                for h in range(HEADS_LOCAL):
                    nc.vector.tensor_scalar(
                        z_n[:, h, :],
                        zps[:, qs, h * 65 : h * 65 + D_HEAD],
                        recip[:, h : h + 1],
                        None,
                        ALU.mult,
                    )
                nc.sync.dma_start_transpose(
                    z_nT[:, bass.ts(sb, P)], z_n[:].rearrange("p h d -> p (h d)")
                )

            def emit_oproj_sb(qt, sbl, fast=False):
                sb = qt * QSUB + sbl
                ops = spool.tile([P, D_MODEL], F32, name="ops", tag="sps")
                for half in range(2):
                    nc.tensor.matmul(
                        ops[:, bass.ts(half, QW)],
                        lhsT=z_nT[:, bass.ts(sb, P)],
                        rhs=wot_sb[:, bass.ts(half, QW)],
                        start=True,
                        stop=True,
                    )
                ot = osb.tile([P, D_MODEL], F32, name="ot", tag="ot")
                if fast:
                    # tail path: split the drain/store across DVE+ACT and
                    # SP+ACT DMA queues to halve the end-of-kernel latency
                    nc.vector.tensor_copy(ot[:, 0:QW], ops[:, 0:QW])
                    nc.scalar.activation(ot[:, QW:], ops[:, QW:], AF.Copy)
                    nc.sync.dma_start(out[bass.ts(sb, P), 0:QW], ot[:, 0:QW])
                    nc.scalar.dma_start(out[bass.ts(sb, P), QW:], ot[:, QW:])
                else:
                    nc.vector.tensor_copy(ot[:], ops[:])
                    nc.sync.dma_start(out[bass.ts(sb, P), :], ot[:])

            def qt0_step_p2(kb):
                # q-tile 0's deferred last score/exp steps (ACT Exp is fine
                # here: phase 2's activation table is already Exp)
                sps = spool.tile([P, EXP_BATCH, QW], F32, name="sps", tag="sps")
                for h in range(HEADS_LOCAL):
                    nc.tensor.matmul(
                        sps[:, h, :],
                        lhsT=kT2[h * D_HEAD : (h + 1) * D_HEAD, bass.ts(kb, P)],
                        rhs=qT2[h * D_HEAD : (h + 1) * D_HEAD, 0:QW],
                        start=True,
                        stop=True,
                    )
                nc.scalar.activation(
                    rings[0][:, 2 * kb : 2 * kb + 2, :], sps[:], AF.Exp
                )

            # deferred-work queue: each q-tile's loop drains `pending` (prior
            # q-tile finalize) plus its own h0 PV waves, spread evenly
            pending = (
                [lambda kb=kb: qt0_step_p2(kb) for kb in qt0_rest]
                + [lambda qs=qs: pv_wave(0, qs, 0) for qs in range(QSUB)]
                + [lambda qs=qs: pv_wave(0, qs, 1) for qs in range(QSUB)]
                + [lambda qs=qs: norm_dve_qs(0, qs) for qs in range(QSUB)]
                + [lambda qs=qs: emit_oproj_sb(0, qs) for qs in range(QSUB)]
            )

            for qt in range(1, QT):
                prev = qt - 1
                qsl = bass.ts(qt, QW)
                ring = rings[qt % 2]
                # h-major tiles: h0's ring slots complete by mid-loop so this
                # q-tile's own h0 PV waves can run before the loop ends
                tiles = [(kb, h) for h in range(HEADS_LOCAL) for kb in range(KB)]
                items = pending + [
                    lambda qs=qs, qt=qt: pv_wave(qt, qs, 0) for qs in range(QSUB)
                ]
                own_start = len(pending)
                n_batches = (len(tiles) + EXP_BATCH - 1) // EXP_BATCH
                slots = {}
                for i in range(len(items)):
                    # own h0 waves must wait for this q-tile's h0 exps
                    lo = (
                        int(i * n_batches / len(items))
                        if i < own_start
                        else max(n_batches - 2 * (len(items) - i), n_batches // 2 + 2)
                    )
                    slots.setdefault(lo, []).append(items[i])

                for bidx, b0 in enumerate(range(0, len(tiles), EXP_BATCH)):
                    batch = tiles[b0 : b0 + EXP_BATCH]
                    nb = len(batch)
                    h = batch[0][1]
                    kb0 = batch[0][0]
                    sps = spool.tile([P, EXP_BATCH, QW], F32, name="sps", tag="sps")
                    for j, (kb, hh) in enumerate(batch):
                        nc.tensor.matmul(
                            sps[:, j, :],
                            lhsT=kT2[
                                hh * D_HEAD : (hh + 1) * D_HEAD, bass.ts(kb, P)
                            ],
                            rhs=qT2[hh * D_HEAD : (hh + 1) * D_HEAD, qsl],
                            start=True,
                            stop=True,
                        )
                    dst = ring[:, 2 * kb0 + h : 2 * (kb0 + nb - 1) + h + 1 : 2, :]
                    if schraud_p2 and bidx % 3 == 2:
                        # exp via int16 bit-trick on DVE (offloads ACT)
                        nc.vector.tensor_scalar(
                            dst.bitcast(I16),
                            sps[:, 0:nb, :],
                            SCHRAUD_A,
                            SCHRAUD_B,
                            ALU.mult,
                            ALU.add,
                        )
                    else:
                        nc.scalar.activation(dst, sps[:, 0:nb, :], AF.Exp)
                    for fn in slots.get(bidx, []):
                        fn()

                pending = (
                    [lambda qs=qs, p=qt: pv_wave(p, qs, 1) for qs in range(QSUB)]
                    + [lambda qs=qs, p=qt: norm_dve_qs(p, qs) for qs in range(QSUB)]
                    + [lambda qs=qs, p=qt: emit_oproj_sb(p, qs) for qs in range(QSUB)]
                )

            # tail: drain the last q-tile's finalize, interleaved per qsub so
            # the PE wave / DVE normalize / PE O-proj stages pipeline
            last = QT - 1
            pv_wave(last, 0, 1)
            for qs in range(QSUB):
                if qs + 1 < QSUB:
                    pv_wave(last, qs + 1, 1)
                norm_dve_qs(last, qs)
                emit_oproj_sb(last, qs, fast=True)

    if split_waits:
        _split_excess_waits(nc)
    return nc


def shard_inputs(x, Wqkv, bqkv, Wo, bo, wq, wk):
    x2 = np.ascontiguousarray(np.asarray(x, dtype=np.float32).reshape(SEQ, D_MODEL))
    Wqkv = np.asarray(Wqkv, dtype=np.float32)
    bqkv = np.asarray(bqkv, dtype=np.float32)
    Wo = np.asarray(Wo, dtype=np.float32)
    wq = np.asarray(wq, dtype=np.float32)
    wk = np.asarray(wk, dtype=np.float32)

    xta0 = np.zeros((DM_AUG, SEQ), np.float16)
    xta0[:D_MODEL] = x2.T.astype(np.float16)
    xta0[D_MODEL] = 1.0
    # tile to [sb, p, c, s'] so each partition's per-s-block line is contiguous
    xta = np.ascontiguousarray(
        xta0.reshape(NCH, P, SB, P).transpose(2, 1, 0, 3)
    )

    # wq*wk is a diagonal metric inside the q.k contraction; applied to the
    # transposed q rows [(q0 d), (q1 d)]
    w2 = (wq * wk).astype(np.float32)
    wqwk2 = np.concatenate([w2, w2]).reshape(P, 1)

    in_maps = []
    for c in range(N_CORES):
        rows, brows = [], []
        for part in range(3):
            for h in (HEADS_LOCAL * c, HEADS_LOCAL * c + 1):
                sl = slice(
                    part * D_MODEL + h * D_HEAD, part * D_MODEL + (h + 1) * D_HEAD
                )
                rows.append(Wqkv[sl])
                brows.append(bqkv[sl])
        Wl = np.concatenate(rows, 0)          # [384, 1024]
        bl = np.concatenate(brows, 0)         # [384]
        wqkvta = np.zeros((DM_AUG, F_LOCAL), np.float16)
        wqkvta[:D_MODEL] = Wl.T.astype(np.float16)
        wqkvta[D_MODEL] = bl.astype(np.float16)
        cols = slice(
            HEADS_LOCAL * c * D_HEAD, (HEADS_LOCAL * c + HEADS_LOCAL) * D_HEAD
        )
        wotc = np.ascontiguousarray(Wo[:, cols].T.astype(np.float16))  # [128, 1024]
        in_maps.append(
            {
                "xta": xta,
                "wqkvt": np.ascontiguousarray(wqkvta),
                "wot": wotc,
                "wqwk": wqwk2,
            }
        )
    return in_maps


_NC_CACHE = {}
LAST_RESULT = None


def kernel(x, Wqkv, bqkv, Wo, bo, wq, wk):
    import os
    from concourse.bass_utils import run_bass_kernel_spmd

    global LAST_RESULT
    assert np.asarray(x).shape == (1, SEQ, D_MODEL)
    in_maps = shard_inputs(x, Wqkv, bqkv, Wo, bo, wq, wk)
    if "nc" not in _NC_CACHE:
        _NC_CACHE["nc"] = build_core_kernel()
    nc = _NC_CACHE["nc"]
    trace = bool(int(os.environ.get("BASS_KERNEL_TRACE", "0")))
    res = run_bass_kernel_spmd(nc, in_maps, list(range(N_CORES)), trace=trace)
    LAST_RESULT = res
    acc = np.zeros((SEQ, D_MODEL), np.float64)
    for c in range(N_CORES):
        acc += res.results[c]["out"].astype(np.float64)
    acc += np.asarray(bo, dtype=np.float64)
    return acc.astype(np.float32).reshape(1, SEQ, D_MODEL)


# revision 4
# speedup vs baseline: 1.0786x; 1.0032x over previous
"""Trainium2 Bass kernel v5 for the 16-head attention block (d_model=1024,
seq=4096), tensor-parallel over heads across 8 cores (2 heads/core).

Dataflow highlights:
  - x / Wqkv / Wo are shipped fp16 (11-bit mantissa ~ fp32r precision for
    these magnitudes); all PE matmuls run at 1.0 cycles/row.
  - q/k are rms-normalized on DVE (rsqrt via pow(-0.5)), cast to fp16, and
    transposed by the DMA XBAR (SBUF->SBUF, ~14ns/tile) -- no PE transposes,
    no PSUM merge traffic. wq*wk (a diagonal metric inside the q.k
    contraction) is folded onto the transposed q rows by the Pool engine.
  - scores: S^T[k,q] fp16 matmuls (free=512); exp on ACT -> bf16 probs;
    roughly every third exp batch is computed on DVE instead via the
    Schraudolph bit-trick (round(S*log2e*2^7 + (127-c)*2^7) as int16 IS
    bf16 exp) so ACT stays below the PE roofline.
  - PV: z[q,65] per head accumulated over k-chunks (lhsT = probs slices,
    rhs = V'[k,65] bf16 with fused ones-column denominators).
  - normalize on DVE -> fp16 z_n, DMA-transposed into z_nT[d,s]; O-proj
    fp16; partial outputs summed on the host (the TP all-reduce).
  - q-tile 0's attention is zigzagged into phase 1; every other q-tile's
    normalize/O-proj is deferred into the next q-tile's batch loop.
"""

import numpy as np
from contextlib import ExitStack

import concourse.bass as bass
import concourse.tile as tile
from concourse import mybir

F32 = mybir.dt.float32
F16 = mybir.dt.float16
BF16 = mybir.dt.bfloat16
I16 = mybir.dt.int16
AF = mybir.ActivationFunctionType
ALU = mybir.AluOpType

D_MODEL = 1024
SEQ = 4096
N_HEADS = 16
D_HEAD = 64
N_CORES = 8
HEADS_LOCAL = 2
P = 128
F_LOCAL = 3 * HEADS_LOCAL * D_HEAD       # 384: [q0|q1|k0|k1|v0|v1]
DM_AUG = D_MODEL + P                     # 1152 rows: x^T plus ones-row block
NCH = DM_AUG // P                        # 9 contraction chunks
SB = SEQ // P                            # 32 s-blocks
QT = 8                                   # q-tiles of 512
QW = SEQ // QT                           # 512
QSUB = QW // P                           # 4 q-subblocks of 128 per q-tile
KB = SEQ // P                            # 32 k-blocks
EXP_BATCH = 2
EPS = 1e-6
# Schraudolph exp constants: int16(S*A + B) bitcast to bf16 ~= exp(S).
# +0.5 makes float->int truncation behave as round-to-nearest; |S| <= 64
# (Cauchy-Schwarz on rms-normalized 64-vectors) keeps y in (0, 32767).
SCHRAUD_A = float(np.log2(np.e) * 128.0)
SCHRAUD_B = float((127.0 - 0.0439) * 128.0 + 0.5)

MAX_WAITS = 1


def _split_excess_waits(nc):
    """This walrus build rejects instructions carrying more than one sync-wait
    command. Rewrite every instruction with more than MAX_WAITS waits into a
    chain of same-engine NoOps each carrying MAX_WAITS waits."""
    import bass_rust

    n_new = 0
    for f in nc.m.functions:
        for bb in f.blocks:
            changed = False
            out = []
            for ins in bb.instructions:
                si = ins.sync_info
                waits = list(si.on_wait) if si is not None and si.on_wait else []
                if len(waits) > MAX_WAITS:
                    changed = True
                    ncar = len(waits) - MAX_WAITS
                    for i in range(0, ncar, MAX_WAITS):
                        chunk = waits[i : min(i + MAX_WAITS, ncar)]
                        nop = mybir.InstNoOp(
                            name=f"{ins.name}-wsplit{i}", ins=[], outs=[]
                        )
                        nop.engine = ins.engine
                        nop.sync_info = bass_rust.SyncInfo(
                            on_wait=chunk, on_update=[]
                        )
                        out.append(nop)
                        n_new += 1
                    ins.sync_info = bass_rust.SyncInfo(
                        on_wait=waits[ncar:], on_update=si.on_update
                    )
                out.append(ins)
            if changed:
                bb.instructions = out
    return n_new


def build_core_kernel(split_waits=True, schraud_qt0=True, schraud_p2=True):
    nc = bass.Bass()
    # x is host-tiled per s-block so each partition's DMA line is one
    # contiguous 2304B run (strided 256B lines halve DMA throughput)
    xta = nc.declare_dram_parameter("xta", [SB, P, NCH, P], F16, isOutput=False)
    wqkvt = nc.declare_dram_parameter("wqkvt", [DM_AUG, F_LOCAL], F16, isOutput=False)
    wot = nc.declare_dram_parameter("wot", [P, D_MODEL], F16, isOutput=False)
    wqwk = nc.declare_dram_parameter("wqwk", [P, 1], F32, isOutput=False)
    out = nc.declare_dram_parameter("out", [SEQ, D_MODEL], F32, isOutput=True)

    wqkvt_r = wqkvt.rearrange("(c p) f -> p c f", p=P)   # [128, 9, 384]

    with ExitStack() as ctx:
        tc = ctx.enter_context(tile.TileContext(nc))

        const = ctx.enter_context(tc.tile_pool(name="const", bufs=1))
        persist = ctx.enter_context(tc.tile_pool(name="persist", bufs=1))

        eps_t = const.tile([P, 1], F32)
        nc.gpsimd.memset(eps_t[:], EPS)
        # const DMAs go out on the ACT queue so the SP queue starts on x
        wqwk_sb = const.tile([P, 1], F32)
        nc.scalar.dma_start(wqwk_sb[:], wqwk[:])
        wqkv_sb = const.tile([P, NCH, F_LOCAL], F16)
        for c in range(NCH):
            nc.scalar.dma_start(wqkv_sb[:, c, :], wqkvt_r[:, c, :])
        # touch the Exp table now so the 1.3us activation-table load overlaps
        # the remaining DMAs instead of stalling the first rms op
        twarm = const.tile([P, 1], F32)
        nc.scalar.activation(twarm[:], wqwk_sb[:], AF.Exp)
        wot_sb = const.tile([P, D_MODEL], F16)
        nc.scalar.dma_start(wot_sb[:], wot[:])

        # transposed q/k: rows (head, d), fp16
        qT2 = persist.tile([P, SEQ], F16)
        kT2 = persist.tile([P, SEQ], F16)
        # V' per head: [k(128), head, kb, 64+1] bf16 with ones column
        vp = persist.tile([P, HEADS_LOCAL, KB, D_HEAD + 1], BF16)
        nc.gpsimd.memset(vp[:, :, :, D_HEAD : D_HEAD + 1], 1.0)
        z_nT = persist.tile([P, SEQ], F16)
        recip = persist.tile([P, 2], F32)
        ss_all = persist.tile([P, SB, 4], F32)
        rs_all = persist.tile([P, SB, 4], F32)
        rr_all = persist.tile([P, SB, 4], F32)
        # two probs retention rings (one per q-tile parity): slot = kb*2 + h
        rings = [
            persist.tile([P, 2 * KB, QW], BF16, name=f"ring{i}") for i in range(2)
        ]

        zps_tiles = {}

        def get_zps(qt):
            t = zpool.tile([P, QSUB, 256], F32, name=f"zps{qt}", tag="zps")
            zps_tiles[qt] = t
            return t

        # ---------------- phase 1 (+ q-tile 0 zigzag) ----------------
        with ExitStack() as p1:
            xpool = p1.enter_context(tc.tile_pool(name="xt", bufs=4))
            norm = p1.enter_context(tc.tile_pool(name="norm", bufs=3))
            qkps = p1.enter_context(tc.tile_pool(name="qkvps", bufs=4, space="PSUM"))
            sp0 = p1.enter_context(tc.tile_pool(name="sps0", bufs=2, space="PSUM"))

            xt_queue = []

            def prefetch_x(sb):
                xt0 = xpool.tile([P, NCH, P], F16)
                nc.sync.dma_start(xt0[:], xta[sb])
                xt_queue.append(xt0)

            def pass1(sb):
                ssl = bass.ts(sb, P)
                xt = xt_queue.pop(0)
                qkv_ps = qkps.tile([P, F_LOCAL], F32)
                # q/k columns first so the rms chain starts before v lands
                for c in range(NCH):
                    nc.tensor.matmul(
                        qkv_ps[:, 0 : 4 * D_HEAD],
                        lhsT=xt[:, c, :],
                        rhs=wqkv_sb[:, c, 0 : 4 * D_HEAD],
                        start=(c == 0),
                        stop=(c == NCH - 1),
                    )
                for c in range(NCH):
                    nc.tensor.matmul(
                        qkv_ps[:, 4 * D_HEAD :],
                        lhsT=xt[:, c, :],
                        rhs=wqkv_sb[:, c, 4 * D_HEAD :],
                        start=(c == 0),
                        stop=(c == NCH - 1),
                        skip_group_check=True,
                    )
                qk_ps = qkv_ps[:, 0 : 4 * D_HEAD].rearrange("p (g d) -> p g d", g=4)
                sq = norm.tile([P, 4, D_HEAD], F32)
                nc.scalar.activation(sq[:], qk_ps, AF.Square)
                nc.vector.tensor_reduce(
                    ss_all[:, sb, :], sq[:], axis=mybir.AxisListType.X, op=ALU.add
                )
                # rsqrt(ms + eps) = Exp(-0.5 * Ln(ms + eps)): Ln/Exp/Square/
                # Copy all live in the natural_log_exp activation table, so
                # the whole kernel runs on a single table load
                nc.scalar.activation(
                    rs_all[:, sb, :],
                    ss_all[:, sb, :],
                    AF.Ln,
                    bias=eps_t[:],
                    scale=1.0 / D_HEAD,
                )
                nc.scalar.activation(
                    rr_all[:, sb, :], rs_all[:, sb, :], AF.Exp, scale=-0.5
                )
                qk_hat = norm.tile([P, 4, D_HEAD], F16)
                nc.vector.tensor_tensor(
                    qk_hat[:],
                    qk_ps,
                    rr_all[:, sb, :, None].to_broadcast((P, 4, D_HEAD)),
                    ALU.mult,
                )
                # V drain on DVE (keeps ACT free for the rms chain + exps)
                nc.vector.tensor_copy(
                    vp[:, :, sb, 0:D_HEAD],
                    qkv_ps[:, 4 * D_HEAD : 6 * D_HEAD].rearrange(
                        "p (h d) -> p h d", h=2
                    ),
                )
                # XBAR transposes; then wq*wk onto the q rows (Pool)
                nc.sync.dma_start_transpose(
                    qT2[:, ssl], qk_hat[:, 0:2, :].rearrange("p g d -> p (g d)")
                )
                nc.sync.dma_start_transpose(
                    kT2[:, ssl], qk_hat[:, 2:4, :].rearrange("p g d -> p (g d)")
                )
                nc.gpsimd.tensor_scalar(
                    qT2[:, ssl], qT2[:, ssl], wqwk_sb[:], None, ALU.mult
                )

            def qt0_step(kb):
                sps0 = sp0.tile([P, HEADS_LOCAL, QW], F32, name="sps0", tag="sps0")
                for h in range(HEADS_LOCAL):
                    nc.tensor.matmul(
                        sps0[:, h, :],
                        lhsT=kT2[h * D_HEAD : (h + 1) * D_HEAD, bass.ts(kb, P)],
                        rhs=qT2[h * D_HEAD : (h + 1) * D_HEAD, 0:QW],
                        start=True,
                        stop=True,
                    )
                # q-tile 0's exps split between ACT (same table as Ln) and
                # DVE Schraudolph to balance the phase-1 engine load
                if kb % 3 == 0:
                    nc.scalar.activation(
                        rings[0][:, 2 * kb : 2 * kb + 2, :], sps0[:], AF.Exp
                    )
                else:
                    nc.vector.tensor_scalar(
                        rings[0][:, 2 * kb : 2 * kb + 2, :].bitcast(I16),
                        sps0[:],
                        SCHRAUD_A,
                        SCHRAUD_B,
                        ALU.mult,
                        ALU.add,
                    )

            for sb in range(4):
                prefetch_x(sb)
            for sb in range(SB):
                if sb + 4 < SB:
                    prefetch_x(sb + 4)
                pass1(sb)
                if sb >= 4:
                    qt0_step(sb - 4)
            # q-tile 0's last steps are deferred into q-tile 1's batch loop
            qt0_rest = list(range(28, KB))

        # ---------- phase 2: q-tiles 1..7 + deferred prev finalize ----------
        with ExitStack() as p2:
            spool = p2.enter_context(tc.tile_pool(name="sps", bufs=3, space="PSUM"))
            # zps PSUM slot shared by all q-tiles ([q(128), qsub, 256-pad]
            # fp32, 2 banks); per head the 65 cols at h*65 = [z | denominator]
            zpool = p2.enter_context(tc.tile_pool(name="zps", bufs=1, space="PSUM"))
            znpool = p2.enter_context(tc.tile_pool(name="zn", bufs=3))
            osb = p2.enter_context(tc.tile_pool(name="osb", bufs=3))

            def pv_wave(qt, qs, h):
                # one (qsub, head) accumulation group, 32 consecutive 65-wide
                # matmuls; groups sharing a PSUM bank never interleave (the
                # hardware marks the whole 2KB bank pending-zero on start)
                if qt not in zps_tiles:
                    get_zps(qt)
                zps = zps_tiles[qt]
                ring = rings[qt % 2]
                for kb in range(KB):
                    nc.tensor.matmul(
                        zps[:, qs, h * 65 : h * 65 + 65],
                        lhsT=ring[:, 2 * kb + h, bass.ts(qs, P)],
                        rhs=vp[:, h, kb, :],
                        start=(kb == 0),
                        stop=(kb == KB - 1),
                        skip_group_check=True,
                    )

            def norm_dve_qs(qt, qs):
                # normalize by the fused denominators; z_n fp16 is
                # XBAR-transposed straight into z_nT[d, s]
                zps = zps_tiles[qt]
                sb = qt * QSUB + qs
                nc.vector.reciprocal(recip[:, 0:2], zps[:, qs, 64 : 64 + 66 : 65])
                z_n = znpool.tile([P, 2, D_HEAD], F16, name="zn", tag="zn")
                for h in range(HEADS_LOCAL):
                    nc.vector.tensor_scalar(
                        z_n[:, h, :],
                        zps[:, qs, h * 65 : h * 65 + D_HEAD],
                        recip[:, h : h + 1],
                        None,
                        ALU.mult,
                    )
                nc.sync.dma_start_transpose(
                    z_nT[:, bass.ts(sb, P)], z_n[:].rearrange("p h d -> p (h d)")
                )

            def emit_oproj_sb(qt, sbl, fast=False):
                sb = qt * QSUB + sbl
                ops = spool.tile([P, D_MODEL], F32, name="ops", tag="sps")
                for half in range(2):
                    nc.tensor.matmul(
                        ops[:, bass.ts(half, QW)],
                        lhsT=z_nT[:, bass.ts(sb, P)],
                        rhs=wot_sb[:, bass.ts(half, QW)],
                        start=True,
                        stop=True,
                    )
                ot = osb.tile([P, D_MODEL], F32, name="ot", tag="ot")
                if fast:
                    # tail path: split the drain/store across DVE+ACT and
                    # SP+ACT DMA queues to halve the end-of-kernel latency
                    nc.vector.tensor_copy(ot[:, 0:QW], ops[:, 0:QW])
                    nc.scalar.activation(ot[:, QW:], ops[:, QW:], AF.Copy)
                    nc.sync.dma_start(out[bass.ts(sb, P), 0:QW], ot[:, 0:QW])
                    nc.scalar.dma_start(out[bass.ts(sb, P), QW:], ot[:, QW:])
                else:
                    nc.vector.tensor_copy(ot[:], ops[:])
                    nc.sync.dma_start(out[bass.ts(sb, P), :], ot[:])

            def qt0_step_p2(kb):
                # q-tile 0's deferred last score/exp steps (ACT Exp is fine
                # here: phase 2's activation table is already Exp)
                sps = spool.tile([P, EXP_BATCH, QW], F32, name="sps", tag="sps")
                for h in range(HEADS_LOCAL):
                    nc.tensor.matmul(
                        sps[:, h, :],
                        lhsT=kT2[h * D_HEAD : (h + 1) * D_HEAD, bass.ts(kb, P)],
                        rhs=qT2[h * D_HEAD : (h + 1) * D_HEAD, 0:QW],
                        start=True,
                        stop=True,
                    )
                nc.scalar.activation(
                    rings[0][:, 2 * kb : 2 * kb + 2, :], sps[:, 0:2, :], AF.Exp
                )

            # deferred-work queue: each q-tile's loop drains `pending` (prior
            # q-tile finalize) plus its own h0 PV waves, spread evenly
            pending = (
                [lambda kb=kb: qt0_step_p2(kb) for kb in qt0_rest]
                + [lambda qs=qs: pv_wave(0, qs, 0) for qs in range(QSUB)]
                + [lambda qs=qs: pv_wave(0, qs, 1) for qs in range(QSUB)]
                + [lambda qs=qs: norm_dve_qs(0, qs) for qs in range(QSUB)]
                + [lambda qs=qs: emit_oproj_sb(0, qs) for qs in range(QSUB)]
            )

            for qt in range(1, QT):
                prev = qt - 1
                qsl = bass.ts(qt, QW)
                ring = rings[qt % 2]
                # h-major batches (never crossing the head boundary): h0's
                # ring slots complete by mid-loop so this q-tile's own h0 PV
                # waves can run before the loop ends
                batches = [
                    [(kb, h) for kb in range(kb0, min(kb0 + EXP_BATCH, KB))]
                    for h in range(HEADS_LOCAL)
                    for kb0 in range(0, KB, EXP_BATCH)
                ]
                items = pending + [
                    lambda qs=qs, qt=qt: pv_wave(qt, qs, 0) for qs in range(QSUB)
                ]
                own_start = len(pending)
                n_batches = len(batches)
                slots = {}
                for i in range(len(items)):
                    # own h0 waves must wait for this q-tile's h0 exps
                    lo = (
                        int(i * n_batches / len(items))
                        if i < own_start
                        else max(n_batches - 2 * (len(items) - i), n_batches // 2 + 2)
                    )
                    slots.setdefault(lo, []).append(items[i])

                for bidx, batch in enumerate(batches):
                    nb = len(batch)
                    h = batch[0][1]
                    kb0 = batch[0][0]
                    sps = spool.tile([P, EXP_BATCH, QW], F32, name="sps", tag="sps")
                    for j, (kb, hh) in enumerate(batch):
                        nc.tensor.matmul(
                            sps[:, j, :],
                            lhsT=kT2[
                                hh * D_HEAD : (hh + 1) * D_HEAD, bass.ts(kb, P)
                            ],
                            rhs=qT2[hh * D_HEAD : (hh + 1) * D_HEAD, qsl],
                            start=True,
                            stop=True,
                        )
                    dst = ring[:, 2 * kb0 + h : 2 * (kb0 + nb - 1) + h + 1 : 2, :]
                    if schraud_p2 and bidx % 3 == 2:
                        # exp via int16 bit-trick on DVE (offloads ACT)
                        nc.vector.tensor_scalar(
                            dst.bitcast(I16),
                            sps[:, 0:nb, :],
                            SCHRAUD_A,
                            SCHRAUD_B,
                            ALU.mult,
                            ALU.add,
                        )
                    else:
                        nc.scalar.activation(dst, sps[:, 0:nb, :], AF.Exp)
                    for fn in slots.get(bidx, []):
                        fn()

                pending = (
                    [lambda qs=qs, p=qt: pv_wave(p, qs, 1) for qs in range(QSUB)]
                    + [lambda qs=qs, p=qt: norm_dve_qs(p, qs) for qs in range(QSUB)]
                    + [lambda qs=qs, p=qt: emit_oproj_sb(p, qs) for qs in range(QSUB)]
                )

            # tail: drain the last q-tile's finalize, interleaved per qsub so
            # the PE wave / DVE normalize / PE O-proj stages pipeline
            last = QT - 1
            pv_wave(last, 0, 1)
            for qs in range(QSUB):
                if qs + 1 < QSUB:
                    pv_wave(last, qs + 1, 1)
                norm_dve_qs(last, qs)
                emit_oproj_sb(last, qs, fast=True)

    if split_waits:
        _split_excess_waits(nc)
    return nc


def shard_inputs(x, Wqkv, bqkv, Wo, bo, wq, wk):
    x2 = np.ascontiguousarray(np.asarray(x, dtype=np.float32).reshape(SEQ, D_MODEL))
    Wqkv = np.asarray(Wqkv, dtype=np.float32)
    bqkv = np.asarray(bqkv, dtype=np.float32)
    Wo = np.asarray(Wo, dtype=np.float32)
    wq = np.asarray(wq, dtype=np.float32)
    wk = np.asarray(wk, dtype=np.float32)

    xta0 = np.zeros((DM_AUG, SEQ), np.float16)
    xta0[:D_MODEL] = x2.T.astype(np.float16)
    xta0[D_MODEL] = 1.0
    # tile to [sb, p, c, s'] so each partition's per-s-block line is contiguous
    xta = np.ascontiguousarray(
        xta0.reshape(NCH, P, SB, P).transpose(2, 1, 0, 3)
    )

    # wq*wk is a diagonal metric inside the q.k contraction; applied to the
    # transposed q rows [(q0 d), (q1 d)]
    w2 = (wq * wk).astype(np.float32)
    wqwk2 = np.concatenate([w2, w2]).reshape(P, 1)

    in_maps = []
    for c in range(N_CORES):
        rows, brows = [], []
        for part in range(3):
            for h in (HEADS_LOCAL * c, HEADS_LOCAL * c + 1):
                sl = slice(
                    part * D_MODEL + h * D_HEAD, part * D_MODEL + (h + 1) * D_HEAD
                )
                rows.append(Wqkv[sl])
                brows.append(bqkv[sl])
        Wl = np.concatenate(rows, 0)          # [384, 1024]
        bl = np.concatenate(brows, 0)         # [384]
        wqkvta = np.zeros((DM_AUG, F_LOCAL), np.float16)
        wqkvta[:D_MODEL] = Wl.T.astype(np.float16)
        wqkvta[D_MODEL] = bl.astype(np.float16)
        cols = slice(
            HEADS_LOCAL * c * D_HEAD, (HEADS_LOCAL * c + HEADS_LOCAL) * D_HEAD
        )
        wotc = np.ascontiguousarray(Wo[:, cols].T.astype(np.float16))  # [128, 1024]
        in_maps.append(
            {
                "xta": xta,
                "wqkvt": np.ascontiguousarray(wqkvta),
                "wot": wotc,
                "wqwk": wqwk2,
            }
        )
    return in_maps


_NC_CACHE = {}
LAST_RESULT = None


def kernel(x, Wqkv, bqkv, Wo, bo, wq, wk):
    import os
    from concourse.bass_utils import run_bass_kernel_spmd

    global LAST_RESULT
    assert np.asarray(x).shape == (1, SEQ, D_MODEL)
    in_maps = shard_inputs(x, Wqkv, bqkv, Wo, bo, wq, wk)
    if "nc" not in _NC_CACHE:
        _NC_CACHE["nc"] = build_core_kernel()
    nc = _NC_CACHE["nc"]
    trace = bool(int(os.environ.get("BASS_KERNEL_TRACE", "0")))
    res = run_bass_kernel_spmd(nc, in_maps, list(range(N_CORES)), trace=trace)
    LAST_RESULT = res
    acc = np.zeros((SEQ, D_MODEL), np.float64)
    for c in range(N_CORES):
        acc += res.results[c]["out"].astype(np.float64)
    acc += np.asarray(bo, dtype=np.float64)
    return acc.astype(np.float32).reshape(1, SEQ, D_MODEL)
